# revision 1
# baseline (speedup 1.0000x reference)
"""Causal self-attention block (QKV proj + causal MHA + out proj + residual
+ LayerNorm) for B=4, S=2048, HID=1024, 16 heads, on 8 Trainium2 cores.

Sharding: core c handles batch b=c//2 and heads [8h, 8h+8) where h=c%2
(Megatron-style head split within a batch pair). Each core computes its 8
heads' attention and a partial output projection over the full 2048 rows;
the two cores of a batch pair combine partials with pairwise
ReduceScatters (chunked, pipelined with compute), then each core applies
residual + LayerNorm to its quarter-rows and returns [1024, 1024].

All matmuls run in float32r (TF32-like, ~11 mantissa bits, 1 cycle/row at
moving-dim>=256). Attention uses the transposed-score layout
(scoresT[sk, sq]): softmax sums fall out of the PV matmul via an appended
ones-row on V, causal structure shrinks above-diagonal tiles, and each
head pair shares fused two-bank PSUM tiles so one ACT exp covers both
heads. ScalarE runs exp only; all PSUM evacuation goes through VectorE.
"""

import numpy as np

import concourse.bacc as bacc
import concourse.mybir as mybir
import concourse.tile as tile
from concourse.bass_utils import run_bass_kernel_spmd

F32 = mybir.dt.float32
F32R = mybir.dt.float32r
AF = mybir.ActivationFunctionType
OP = mybir.AluOpType

N_CORES = 8
B, S, HID = 4, 2048, 1024
NHC = 8          # heads per core
DH = 64          # head dim
HW = 512         # per-core head width (NHC * DH)
SQT = 512        # sq tile width
NSQT = S // SQT  # 4
NHCH = HID // 128  # 8 hid chunks
SH = S // 2      # rows per core in the epilogue
EPS = 1e-5

_CACHE = {}


def _build():
    nc = bacc.Bacc("TRN2", target_bir_lowering=False, debug=False,
                   num_devices=N_CORES)

    xT = nc.dram_tensor("xT", [HID, S], F32R, kind="ExternalInput").ap()
    xh = nc.dram_tensor("xh", [SH, HID], F32, kind="ExternalInput").ap()
    wqT = nc.dram_tensor("wqT", [HID, HW], F32R, kind="ExternalInput").ap()
    wkT = nc.dram_tensor("wkT", [HID, HW], F32R, kind="ExternalInput").ap()
    wvT = nc.dram_tensor("wvT", [HID, HW], F32R, kind="ExternalInput").ap()
    woT = nc.dram_tensor("woT", [HW, HID], F32R, kind="ExternalInput").ap()
    bq4 = nc.dram_tensor("bq4", [128, 4], F32, kind="ExternalInput").ap()
    bk4 = nc.dram_tensor("bk4", [128, 4], F32, kind="ExternalInput").ap()
    bvb = nc.dram_tensor("bvb", [128, HW], F32, kind="ExternalInput").ap()
    gmb = nc.dram_tensor("gmb", [128, HID], F32, kind="ExternalInput").ap()
    btb = nc.dram_tensor("btb", [128, HID], F32, kind="ExternalInput").ap()
    m128 = nc.dram_tensor("m128", [128, 128], F32, kind="ExternalInput").ap()
    vone = nc.dram_tensor("vone", [128, 8], F32R, kind="ExternalInput").ap()
    one64 = nc.dram_tensor("one64", [1, 64], F32R, kind="ExternalInput").ap()

    out = nc.dram_tensor("out", [SH, HID], F32, kind="ExternalOutput").ap()

    po_d = nc.dram_tensor("po_d", [S, HID], F32)
    rs_d = nc.dram_tensor("rs_d", [SH, HID], F32)

    from contextlib import ExitStack
    with tile.TileContext(nc) as tc, ExitStack() as es:
        TP = tc.tile_pool
        cp = es.enter_context(TP(name="consts", bufs=1))
        ktp = es.enter_context(TP(name="kt", bufs=1))
        vtp = es.enter_context(TP(name="vt", bufs=1))
        wop = es.enter_context(TP(name="wo", bufs=1))
        ep = es.enter_context(TP(name="exp", bufs=2))
        atp = es.enter_context(TP(name="att", bufs=1))
        avp = es.enter_context(TP(name="av", bufs=2))
        rp = es.enter_context(TP(name="rcp", bufs=2))
        poep = es.enter_context(TP(name="poe", bufs=1))
        pp = es.enter_context(TP(name="pp", bufs=2, space="PSUM"))
        sp = es.enter_context(TP(name="sp", bufs=2, space="PSUM"))
        app = es.enter_context(TP(name="ap", bufs=1, space="PSUM"))
        if True:

            # ---- weights first (QKV needed immediately) ----
            wq, wk, wv = [], [], []
            # (filled below once the wqkv pool exists)

            # ---- constants ----
            mask = cp.tile([128, 128], F32)
            nc.sync.dma_start(mask[:], m128[:])
            bqs = cp.tile([128, 4], F32)
            nc.sync.dma_start(bqs[:], bq4[:])
            bks = cp.tile([128, 4], F32)
            nc.sync.dma_start(bks[:], bk4[:])
            bvs = cp.tile([128, HW], F32)
            nc.sync.dma_start(bvs[:], bvb[:])
            vos = cp.tile([128, 8], F32R)
            nc.sync.dma_start(vos[:], vone[:])
            epsc = cp.tile([128, 1], F32)
            nc.vector.memset(epsc[:], EPS)
            o64 = cp.tile([1, 64], F32R)
            nc.sync.dma_start(o64[:], one64[:])
            gms = cp.tile([128, HID], F32)
            bts = cp.tile([128, HID], F32)

            wot = [wop.tile([128, HID], F32R, name=f"wo{d}")
                   for d in range(4)]

            kt = [ktp.tile([128, S], F32R, name=f"kt{p}") for p in range(4)]
            vt = [vtp.tile([128, 8, 65], F32R, name=f"vt{i}")
                  for i in range(16)]

            wp = es.enter_context(TP(name="wqkv", bufs=1))
            xp = es.enter_context(TP(name="xts", bufs=1))
            qtp = es.enter_context(TP(name="qt", bufs=1))
            lp = es.enter_context(TP(name="ln", bufs=1))
            lsp = es.enter_context(TP(name="lns", bufs=2))
            if True:

                wq, wk, wv = [], [], []
                for nm, dr, lst in (("wq", wqT, wq), ("wk", wkT, wk),
                                    ("wv", wvT, wv)):
                    for hh in range(NHCH):
                        w = wp.tile([128, HW], F32R, name=f"{nm}{hh}")
                        nc.sync.dma_start(w[:], dr[128 * hh:128 * (hh + 1), :])
                        lst.append(w)

                def emit_ln(c_):
                    """Residual + LayerNorm for output chunk c_ (128 rows)."""
                    rs = lp.tile([128, HID], F32, tag="rs")
                    nc.gpsimd.dma_start(rs[:],
                                        rs_d[128 * c_:128 * (c_ + 1), :])
                    xc = lp.tile([128, HID], F32, tag="xc")
                    nc.gpsimd.dma_start(xc[:],
                                        xh[128 * c_:128 * (c_ + 1), :])
                    nc.vector.tensor_add(rs[:], rs[:], xc[:])
                    st6 = lsp.tile([128, 12], F32, tag="st6")
                    nc.vector.bn_stats(st6[:, 0:6], rs[:, 0:512])
                    nc.vector.bn_stats(st6[:, 6:12], rs[:, 512:1024])
                    mv = lsp.tile([128, 2], F32, tag="mv")
                    nc.vector.bn_aggr(mv[:], st6[:])
                    sd = lsp.tile([128, 1], F32, tag="sd")
                    nc.scalar.activation(sd[:], mv[:, 1:2], AF.Sqrt,
                                         bias=epsc[:])
                    inv = lsp.tile([128, 1], F32, tag="inv")
                    nc.vector.reciprocal_approx_fast(inv[:], sd[:])
                    nc.vector.tensor_scalar(xc[:], rs[:], mv[:, 0:1], inv[:],
                                            op0=OP.subtract, op1=OP.mult)
                    nc.vector.tensor_mul(rs[:], xc[:], gms[:])
                    nc.vector.tensor_add(rs[:], rs[:], bts[:])
                    nc.gpsimd.dma_start(out[128 * c_:128 * (c_ + 1), :],
                                        rs[:])

                ln_inited = [False]

                def ln_consts():
                    if not ln_inited[0]:
                        ln_inited[0] = True
                        nc.sync.dma_start(gms[:], gmb[:])
                        nc.sync.dma_start(bts[:], btb[:])

                for t in range(NSQT):
                    # ---- LayerNorm delayed by two tiles ----
                    if t > 1:
                        ln_consts()
                        emit_ln(2 * (t - 2))
                        emit_ln(2 * (t - 2) + 1)
                    # ---- phase A: projections for sq tile t ----
                    xts = []
                    for hh in range(NHCH):
                        xt_ = xp.tile([128, SQT], F32R, tag=f"xt{hh}")
                        nc.sync.dma_start(
                            xt_[:], xT[128 * hh:128 * (hh + 1),
                                       SQT * t:SQT * (t + 1)])
                        xts.append(xt_)

                    qts = []
                    for m in range(4):
                        ps = pp.tile([128, SQT], F32, tag="pq")
                        for hh in range(NHCH):
                            nc.tensor.matmul(
                                ps[:], wq[hh][:, 128 * m:128 * (m + 1)],
                                xts[hh][:], start=(hh == 0),
                                stop=(hh == NHCH - 1))
                        qt_ = qtp.tile([128, SQT], F32R, tag=f"q{m}")
                        nc.vector.tensor_scalar_add(qt_[:], ps[:],
                                                    bqs[:, m:m + 1])
                        qts.append(qt_)
                    for m in range(4):
                        ps = pp.tile([128, SQT], F32, tag="pq")
                        for hh in range(NHCH):
                            nc.tensor.matmul(
                                ps[:], wk[hh][:, 128 * m:128 * (m + 1)],
                                xts[hh][:], start=(hh == 0),
                                stop=(hh == NHCH - 1))
                        nc.vector.tensor_scalar_add(
                            kt[m][:, SQT * t:SQT * (t + 1)], ps[:],
                            bks[:, m:m + 1])
                    for s_ in range(4):
                        i = 4 * t + s_
                        ps = pp.tile([128, HW], F32, tag="pq")
                        for hh in range(NHCH):
                            nc.tensor.matmul(
                                ps[:], xts[hh][:, 128 * s_:128 * (s_ + 1)],
                                wv[hh][:], start=(hh == 0),
                                stop=(hh == NHCH - 1))
                        nc.vector.tensor_tensor(
                            vt[i][:, :, 0:64], ps[:], bvs[:], op=OP.add)
                        nc.vector.tensor_copy(vt[i][:, :, 64:65], vos[:])

                    # ---- phase B: attention for sq tile j = t ----
                    j = t
                    at_tiles = []
                    for p in range(4):
                        pv2 = app.tile([128, 2 * SQT], F32, tag="pv2")
                        for i in range(4 * j + 4):
                            d = i - 4 * j
                            lo_qk = min(128 * d, 256) if d >= 0 else 0
                            lo = 128 * d if d >= 0 else 0
                            s2 = sp.tile([128, 2 * SQT], F32, tag="s2")
                            nc.tensor.matmul(
                                s2[:, lo_qk:SQT],
                                kt[p][0:64, 128 * i:128 * (i + 1)],
                                qts[p][0:64, lo_qk:SQT],
                                start=True, stop=True, tile_position=(0, 0))
                            nc.tensor.matmul(
                                s2[:, SQT + lo_qk:2 * SQT],
                                kt[p][64:128, 128 * i:128 * (i + 1)],
                                qts[p][64:128, lo_qk:SQT],
                                start=True, stop=True, tile_position=(64, 0))
                            e2 = ep.tile([128, 2 * SQT], F32R, tag="e2")
                            s2v = s2[:].rearrange("p (a b) -> p a b", a=2)
                            e2v = e2[:].rearrange("p (a b) -> p a b", a=2)
                            nc.scalar.activation(e2v[:, :, lo:SQT],
                                                 s2v[:, :, lo:SQT],
                                                 AF.Exp, scale=0.125)
                            if d >= 0:
                                nc.vector.tensor_mul(
                                    e2[:, lo:lo + 128], e2[:, lo:lo + 128],
                                    mask[:])
                                nc.vector.tensor_mul(
                                    e2[:, SQT + lo:SQT + lo + 128],
                                    e2[:, SQT + lo:SQT + lo + 128], mask[:])
                            nc.tensor.matmul(
                                pv2[0:65, lo:SQT],
                                vt[i][:, 2 * p, :], e2[:, lo:SQT],
                                start=(i == 0), stop=(i == 4 * j + 3))
                            nc.tensor.matmul(
                                pv2[0:65, SQT + lo:2 * SQT],
                                vt[i][:, 2 * p + 1, :],
                                e2[:, SQT + lo:2 * SQT],
                                start=(i == 0), stop=(i == 4 * j + 3))
                        # quick PSUM evac, then per-pair normalize
                        av2 = avp.tile([65, 2 * SQT], F32, tag="av")
                        nc.vector.tensor_copy(av2[:], pv2[0:65, :])
                        at_ = atp.tile([128, SQT], F32R, tag=f"at{p}")
                        for hb in range(2):
                            sm = rp.tile([1, SQT], F32, tag="sm", bufs=1)
                            nc.vector.tensor_copy(
                                sm[:], av2[64:65, SQT * hb:SQT * (hb + 1)])
                            rc = rp.tile([1, SQT], F32, tag="rc", bufs=1)
                            nc.vector.reciprocal_approx_fast(rc[:], sm[:])
                            rb = rp.tile([64, SQT], F32, tag="rb", bufs=1)
                            nc.gpsimd.partition_broadcast(rb[:], rc[:])
                            nc.vector.tensor_mul(
                                at_[64 * hb:64 * (hb + 1), :],
                                av2[0:64, SQT * hb:SQT * (hb + 1)],
                                rb[:])
                        at_tiles.append(at_)
                        if t == 3 and p == 1:
                            ln_consts()
                            emit_ln(4)
                            emit_ln(5)

                    # ---- phase C: out projection for sq tile j ----
                    if t == 0:
                        for d in range(4):
                            nc.sync.dma_start(
                                wot[d][:], woT[128 * d:128 * (d + 1), :])
                    for c_ in range(4):
                        po = poep.tile([128, HID], F32, tag="po")
                        for o in range(2):
                            ps = pp.tile([128, SQT], F32, tag="pq")
                            for dch in range(4):
                                nc.tensor.matmul(
                                    ps[:],
                                    at_tiles[dch][:, 128 * c_:128 * (c_ + 1)],
                                    wot[dch][:, SQT * o:SQT * (o + 1)],
                                    start=(dch == 0), stop=(dch == 3))
                            nc.vector.tensor_copy(
                                po[:, SQT * o:SQT * (o + 1)], ps[:])
                        r0 = SQT * j + 128 * c_
                        nc.sync.dma_start(po_d[r0:r0 + 128, :], po[:])
                        if c_ in (1, 3):
                            h0 = SQT * j + 256 * (c_ // 2)
                            k = 2 * j + c_ // 2
                            nc.gpsimd.collective_compute(
                                "ReduceScatter",
                                OP.add,
                                replica_groups=[[0, 1], [2, 3],
                                                [4, 5], [6, 7]],
                                ins=[po_d[h0:h0 + 256, :]],
                                outs=[rs_d[128 * k:128 * (k + 1), :]],
                            )


                for jj in (NSQT - 2, NSQT - 1):
                    emit_ln(2 * jj)
                    emit_ln(2 * jj + 1)


    nc.compile()
    return nc


def _prep_inputs(x, Wq, bq, Wk, bk, Wv, bv, Wo, bo, gamma, beta):
    """Shard + lay out the full inputs for the 8 cores."""
    f32 = np.float32
    x = np.asarray(x, f32)
    Wq, bq = np.asarray(Wq, f32), np.asarray(bq, f32)
    Wk, bk = np.asarray(Wk, f32), np.asarray(bk, f32)
    Wv, bv = np.asarray(Wv, f32), np.asarray(bv, f32)
    Wo, bo = np.asarray(Wo, f32), np.asarray(bo, f32)
    gamma, beta = np.asarray(gamma, f32), np.asarray(beta, f32)

    mask = np.triu(np.ones((128, 128), f32))
    vone = np.ones((128, 8), f32)
    gmb = np.ascontiguousarray(np.broadcast_to(gamma, (128, HID)))
    btb = np.ascontiguousarray(np.broadcast_to(beta, (128, HID)))

    halves = []
    for h in range(2):
        sl = slice(HW * h, HW * (h + 1))
        halves.append(dict(
            wqT=np.ascontiguousarray(Wq.T[:, sl]),
            wkT=np.ascontiguousarray(Wk.T[:, sl]),
            wvT=np.ascontiguousarray(Wv.T[:, sl]),
            woT=np.ascontiguousarray(Wo[:, sl].T),
            bq4=np.ascontiguousarray(bq[sl].reshape(4, 128).T),
            bk4=np.ascontiguousarray(bk[sl].reshape(4, 128).T),
            bvb=np.ascontiguousarray(np.broadcast_to(bv[sl], (128, HW))),
        ))

    in_maps = []
    for c in range(N_CORES):
        b, h = c // 2, c % 2
        m = dict(halves[h])
        m["xT"] = np.ascontiguousarray(x[b].T)
        # rows this core receives from the chunked pairwise RS:
        # chunk j covers global rows [512j + 256h, 512j + 256h + 256)
        m["xh"] = np.ascontiguousarray(
            np.concatenate([x[b, 256 * k + 128 * h:256 * k + 128 * h + 128, :]
                            for k in range(8)], axis=0) + bo)
        m["gmb"] = gmb
        m["btb"] = btb
        m["m128"] = mask
        m["vone"] = vone
        m["one64"] = np.ones((1, 64), f32)
        in_maps.append(m)
    return in_maps


def _run(inputs, trace=False):
    if "nc" not in _CACHE:
        _CACHE["nc"] = _build()
    nc = _CACHE["nc"]
    in_maps = _prep_inputs(**inputs)
    res = run_bass_kernel_spmd(nc, in_maps, list(range(N_CORES)),
                               trace=trace)
    out = np.empty((B, S, HID), np.float32)
    for c in range(N_CORES):
        b, h = c // 2, c % 2
        o = res.results[c]["out"]
        for k in range(8):
            out[b, 256 * k + 128 * h:256 * k + 128 * h + 128, :] = \
                o[128 * k:128 * (k + 1), :]
    return out, res


def kernel(**inputs):
    out, _ = _run(inputs, trace=False)
    return out



# revision 5
# speedup vs baseline: 1.3252x; 1.3252x over previous
"""Causal self-attention block (QKV proj + causal MHA + out proj + residual
+ LayerNorm) for B=4, S=2048, HID=1024, 16 heads, on 8 Trainium2 cores.

Sharding: core c handles batch b=c//2 and heads [8h, 8h+8) where h=c%2
(Megatron-style head split within a batch pair). Each core computes its 8
heads' attention and a partial output projection over the full 2048 rows;
the two cores of a batch pair combine partials with pairwise bf16
ReduceScatters (chunked, pipelined with compute), then each core applies
residual + LayerNorm to its quarter-rows and returns [1024, 1024].

All matmuls run in bf16 (fp32 PSUM accumulation). Attention uses the
transposed-score layout (scoresT[sk, sq]): softmax sums fall out of the
PV matmul via an appended ones-row on V, causal structure shrinks
above-diagonal tiles, and each head pair shares fused two-bank PSUM
tiles so one ACT exp covers both heads. The Scalar engine runs exp, the
Q/K/V bias/evac copies, and the LN rsqrt (as exp(-0.5*ln(v)), keeping a
single activation table loaded). Projection work for tile t+1 and the
out projection for tile j-1 are interleaved into attention tile j's
emission to keep the PE dense (p-state) and busy during softmax
normalization windows; each LayerNorm chunk is deferred until well after
its ReduceScatter fires so collective latency never stalls the engines.
"""

import numpy as np
import ml_dtypes

import concourse.bacc as bacc
import concourse.mybir as mybir
import concourse.tile as tile
from concourse.bass_utils import run_bass_kernel_spmd

F32 = mybir.dt.float32
BF16 = mybir.dt.bfloat16
AF = mybir.ActivationFunctionType
OP = mybir.AluOpType
BFNP = ml_dtypes.bfloat16

N_CORES = 8
B, S, HID = 4, 2048, 1024
NHC = 8          # heads per core
DH = 64          # head dim
HW = 512         # per-core head width (NHC * DH)
SQT = 512        # sq tile width
NSQT = S // SQT  # 4
NHCH = HID // 128  # 8 hid chunks
SH = S // 2      # rows per core in the epilogue
EPS = 1e-5
GROUPS = [[0, 1], [2, 3], [4, 5], [6, 7]]

_CACHE = {}


def _build(apply_gb):
    nc = bacc.Bacc("TRN2", target_bir_lowering=False, debug=False,
                   num_devices=N_CORES)

    xT16 = nc.dram_tensor("xT16", [HID, S], BF16, kind="ExternalInput").ap()
    xh = nc.dram_tensor("xh", [SH, HID], F32, kind="ExternalInput").ap()
    wq16 = nc.dram_tensor("wq16", [HID, HW], BF16, kind="ExternalInput").ap()
    wk16 = nc.dram_tensor("wk16", [HID, HW], BF16, kind="ExternalInput").ap()
    wv16 = nc.dram_tensor("wv16", [HID, HW], BF16, kind="ExternalInput").ap()
    wo16 = nc.dram_tensor("wo16", [HW, HID], BF16, kind="ExternalInput").ap()
    bq4 = nc.dram_tensor("bq4", [128, 4], F32, kind="ExternalInput").ap()
    bk4 = nc.dram_tensor("bk4", [128, 4], F32, kind="ExternalInput").ap()
    bv1 = nc.dram_tensor("bv1", [1, HW], BF16, kind="ExternalInput").ap()
    one1 = nc.dram_tensor("one1", [1, 128], BF16, kind="ExternalInput").ap()
    vone = nc.dram_tensor("vone", [128, 8], BF16, kind="ExternalInput").ap()
    m128 = nc.dram_tensor("m128", [128, 128], BF16, kind="ExternalInput").ap()
    gmb = nc.dram_tensor("gmb", [128, HID], F32, kind="ExternalInput").ap()
    btb = nc.dram_tensor("btb", [128, HID], F32, kind="ExternalInput").ap()

    out = nc.dram_tensor("out", [SH, HID], F32, kind="ExternalOutput").ap()

    po_d = nc.dram_tensor("po_d", [S, HID], BF16).ap()
    rs_d = nc.dram_tensor("rs_d", [SH, HID], BF16).ap()

    from contextlib import ExitStack
    with tile.TileContext(nc) as tc, ExitStack() as es:
        TP = tc.tile_pool
        cp = es.enter_context(TP(name="consts", bufs=1))
        xsp = es.enter_context(TP(name="xs", bufs=1))
        wp = es.enter_context(TP(name="w", bufs=1))
        ktp = es.enter_context(TP(name="kt", bufs=1))
        vtp = es.enter_context(TP(name="vt", bufs=1))
        qtp = es.enter_context(TP(name="qt", bufs=2))
        ep = es.enter_context(TP(name="exp", bufs=2))
        atp = es.enter_context(TP(name="att", bufs=2))
        pop = es.enter_context(TP(name="po", bufs=2))
        rp = es.enter_context(TP(name="rcp", bufs=2))
        rbp = es.enter_context(TP(name="rb", bufs=2))
        lp = es.enter_context(TP(name="ln", bufs=2))
        lsp = es.enter_context(TP(name="lns", bufs=2))
        pp = es.enter_context(TP(name="pp", bufs=2, space="PSUM"))
        sp = es.enter_context(TP(name="sp", bufs=2, space="PSUM"))
        app = es.enter_context(TP(name="ap", bufs=1, space="PSUM"))

        # ---- constants ----
        mask = cp.tile([128, 128], BF16)
        nc.sync.dma_start(mask[:], m128[:])
        bqs = cp.tile([128, 4], F32)
        nc.sync.dma_start(bqs[:], bq4[:])
        bks = cp.tile([128, 4], F32)
        nc.sync.dma_start(bks[:], bk4[:])
        bvs = cp.tile([1, HW], BF16)
        nc.sync.dma_start(bvs[:], bv1[:])
        o1s = cp.tile([1, 128], BF16)
        nc.sync.dma_start(o1s[:], one1[:])
        vos = cp.tile([128, 8], BF16)
        nc.sync.dma_start(vos[:], vone[:])
        epsc = cp.tile([128, 1], F32)
        nc.vector.memset(epsc[:], EPS)
        if apply_gb:
            gms = cp.tile([128, HID], F32)
            nc.sync.dma_start(gms[:], gmb[:])
            bts = cp.tile([128, HID], F32)
            nc.sync.dma_start(bts[:], btb[:])

        # ---- persistent tiles + weight/x preload ----
        wq, wk, wv = [], [], []
        for hh in range(NHCH):
            w = wp.tile([128, HW], BF16, name=f"wq{hh}")
            nc.sync.dma_start(w[:], wq16[128 * hh:128 * (hh + 1), :])
            wq.append(w)
        xs = []
        for hh in range(NHCH):
            xt_ = xsp.tile([128, S], BF16, name=f"xs{hh}")
            nc.sync.dma_start(xt_[:], xT16[128 * hh:128 * (hh + 1), :])
            xs.append(xt_)
        for nm, dr, lst in (("wk", wk16, wk), ("wv", wv16, wv)):
            for hh in range(NHCH):
                w = wp.tile([128, HW], BF16, name=f"{nm}{hh}")
                nc.sync.dma_start(w[:], dr[128 * hh:128 * (hh + 1), :])
                lst.append(w)
        wo = []
        for d in range(4):
            w = wp.tile([128, HID], BF16, name=f"wo{d}")
            nc.sync.dma_start(w[:], wo16[128 * d:128 * (d + 1), :])
            wo.append(w)

        kt = [ktp.tile([128, S], BF16, name=f"kt{p}") for p in range(4)]
        vt = [vtp.tile([128, 8, 65], BF16, name=f"vt{i}") for i in range(16)]
        for i in range(16):
            nc.vector.tensor_copy(
                vt[i][:, :, 64:65],
                vos[:].rearrange("p (a b) -> p a b", a=8))

        qts_map = {}
        at_map = {}

        # ---- phase-A units: projections for sq tile t ----
        def unit_q(t, m):
            ps = pp.tile([128, SQT], F32, tag="pq")
            for hh in range(NHCH):
                nc.tensor.matmul(
                    ps[:], wq[hh][:, 128 * m:128 * (m + 1)],
                    xs[hh][:, SQT * t:SQT * (t + 1)],
                    start=(hh == 0), stop=(hh == NHCH - 1))
            qt_ = qtp.tile([128, SQT], BF16, tag=f"q{m}")
            nc.scalar.activation(qt_[:], ps[:], AF.Identity,
                                 bias=bqs[:, m:m + 1])
            qts_map[(t, m)] = qt_

        def unit_k(t, m):
            ps = pp.tile([128, SQT], F32, tag="pq")
            for hh in range(NHCH):
                nc.tensor.matmul(
                    ps[:], wk[hh][:, 128 * m:128 * (m + 1)],
                    xs[hh][:, SQT * t:SQT * (t + 1)],
                    start=(hh == 0), stop=(hh == NHCH - 1))
            nc.scalar.activation(kt[m][:, SQT * t:SQT * (t + 1)], ps[:],
                                 AF.Identity, bias=bks[:, m:m + 1])

        def unit_v(t, s_):
            i = 4 * t + s_
            ps = pp.tile([128, HW], F32, tag="pq")
            c0 = SQT * t + 128 * s_
            for hh in range(NHCH):
                nc.tensor.matmul(
                    ps[:], xs[hh][:, c0:c0 + 128], wv[hh][:],
                    start=(hh == 0), stop=False)
            nc.tensor.matmul(ps[:], o1s[:], bvs[:], start=False, stop=True)
            nc.scalar.activation(
                vt[i][:, :, 0:64],
                ps[:].rearrange("p (a b) -> p a b", a=8), AF.Copy)

        def a_units(t):
            us = []
            for m in range(4):
                us.append(lambda m=m: unit_k(t, m))
            for m in range(4):
                us.append(lambda m=m: unit_q(t, m))
            for s_ in range(4):
                us.append(lambda s_=s_: unit_v(t, s_))
            return us

        # ---- partial out projection for row chunk c of sq tile j ----
        def emit_outproj(j, c):
            at_tiles = [at_map[(j, p)] for p in range(4)]
            po = pop.tile([128, HID], BF16, tag="po")
            for o in range(2):
                ps = pp.tile([128, SQT], F32, tag="pq")
                for dch in range(4):
                    nc.tensor.matmul(
                        ps[:], at_tiles[dch][:, 128 * c:128 * (c + 1)],
                        wo[dch][:, SQT * o:SQT * (o + 1)],
                        start=(dch == 0), stop=(dch == 3))
                nc.vector.tensor_copy(po[:, SQT * o:SQT * (o + 1)], ps[:])
            r0 = SQT * j + 128 * c
            nc.sync.dma_start(po_d[r0:r0 + 128, :], po[:])
            if c in (1, 3):
                h0 = SQT * j + 256 * (c // 2)
                k = 2 * j + c // 2
                nc.gpsimd.collective_compute(
                    "ReduceScatter", OP.add, replica_groups=GROUPS,
                    ins=[po_d[h0:h0 + 256, :]],
                    outs=[rs_d[128 * k:128 * (k + 1), :]])

        # ---- residual + LayerNorm for output chunk k (128 rows) ----
        def emit_ln(k):
            rs = lp.tile([128, HID], BF16, tag="rs")
            nc.gpsimd.dma_start(rs[:], rs_d[128 * k:128 * (k + 1), :])
            xc = lp.tile([128, HID], F32, tag="xc")
            nc.gpsimd.dma_start(xc[:], xh[128 * k:128 * (k + 1), :])
            y = lp.tile([128, HID], F32, tag="y")
            nc.vector.tensor_tensor(y[:], rs[:], xc[:], op=OP.add)
            st6 = lsp.tile([128, 12], F32, tag="st6")
            nc.vector.bn_stats(st6[:, 0:6], y[:, 0:512])
            nc.vector.bn_stats(st6[:, 6:12], y[:, 512:1024])
            mv = lsp.tile([128, 2], F32, tag="mv")
            nc.vector.bn_aggr(mv[:], st6[:])
            lnv = lsp.tile([128, 1], F32, tag="lnv")
            nc.scalar.activation(lnv[:], mv[:, 1:2], AF.Ln, bias=epsc[:])
            inv = lsp.tile([128, 1], F32, tag="inv")
            nc.scalar.activation(inv[:], lnv[:], AF.Exp, scale=-0.5)
            nc.vector.tensor_scalar(xc[:], y[:], mv[:, 0:1], inv[:],
                                    op0=OP.subtract, op1=OP.mult)
            if apply_gb:
                nc.vector.tensor_mul(xc[:], xc[:], gms[:])
                nc.vector.tensor_add(xc[:], xc[:], bts[:])
            nc.gpsimd.dma_start(out[128 * k:128 * (k + 1), :], xc[:])

        # ---- attention p-group for sq tile j ----
        def emit_attn_p(j, p):
            qt_ = qts_map[(j, p)]
            pv2 = app.tile([65, 2 * SQT], F32, tag="pv2")
            last = 4 * j + 3
            for i in range(4 * j + 4):
                d = i - 4 * j
                lo = 128 * d if d >= 0 else 0
                s2 = sp.tile([128, 2 * SQT], F32, tag="s2")
                nc.tensor.matmul(
                    s2[:, lo:SQT],
                    kt[p][0:64, 128 * i:128 * (i + 1)],
                    qt_[0:64, lo:SQT],
                    start=True, stop=True, tile_position=(0, 0))
                nc.tensor.matmul(
                    s2[:, SQT + lo:2 * SQT],
                    kt[p][64:128, 128 * i:128 * (i + 1)],
                    qt_[64:128, lo:SQT],
                    start=True, stop=True, tile_position=(64, 0))
                e2 = ep.tile([128, 2 * SQT], BF16, tag="e2")
                s2v = s2[:].rearrange("p (a b) -> p a b", a=2)
                e2v = e2[:].rearrange("p (a b) -> p a b", a=2)
                nc.scalar.activation(e2v[:, :, lo:SQT], s2v[:, :, lo:SQT],
                                     AF.Exp, scale=0.125)
                if d >= 0:
                    nc.vector.tensor_mul(
                        e2[:, lo:lo + 128], e2[:, lo:lo + 128], mask[:])
                    nc.vector.tensor_mul(
                        e2[:, SQT + lo:SQT + lo + 128],
                        e2[:, SQT + lo:SQT + lo + 128], mask[:])
                nc.tensor.matmul(
                    pv2[:, lo:SQT], vt[i][:, 2 * p, :], e2[:, lo:SQT],
                    start=(i == 0), stop=(i == last))
                nc.tensor.matmul(
                    pv2[:, SQT + lo:2 * SQT], vt[i][:, 2 * p + 1, :],
                    e2[:, SQT + lo:2 * SQT],
                    start=(i == 0), stop=(i == last))
            sm = rp.tile([1, 2 * SQT], F32, tag="sm")
            nc.vector.tensor_copy(sm[:], pv2[64:65, :])
            rc = rp.tile([1, 2 * SQT], F32, tag="rc")
            nc.vector.reciprocal_approx_fast(rc[:], sm[:])
            rb = rbp.tile([64, 2 * SQT], F32, tag="rb")
            nc.gpsimd.partition_broadcast(rb[:], rc[:])
            at_ = atp.tile([128, SQT], BF16, tag=f"at{p}")
            nc.vector.tensor_tensor(at_[0:64, :], pv2[0:64, 0:SQT],
                                    rb[:, 0:SQT], op=OP.mult)
            nc.vector.tensor_tensor(at_[64:128, :], pv2[0:64, SQT:2 * SQT],
                                    rb[:, SQT:2 * SQT], op=OP.mult)
            at_map[(j, p)] = at_

        # ---- emission schedule ----
        # attention(j) interleaves: projection units for tile j+1 (keeps the
        # PE fed through softmax-normalization windows), the out projection
        # for tile j-1 (whose attention output is complete), and deferred
        # LayerNorm chunks whose ReduceScatter fired ~2 p-groups earlier.
        ln_pending = []
        for u in a_units(0):
            u()
        for j in range(NSQT):
            nxt = a_units(j + 1) if j + 1 < NSQT else []
            for p in range(4):
                emit_attn_p(j, p)
                for u in nxt[3 * p:3 * p + 3]:
                    u()
                if j >= 1:
                    if p == 2:
                        emit_outproj(j - 1, 0)
                        emit_outproj(j - 1, 1)
                        ln_pending.append(2 * (j - 1))
                    elif p == 3:
                        emit_outproj(j - 1, 2)
                        emit_outproj(j - 1, 3)
                        ln_pending.append(2 * (j - 1) + 1)
                    elif ln_pending:
                        emit_ln(ln_pending.pop(0))
        for c in range(4):
            emit_outproj(NSQT - 1, c)
            if c == 1:
                ln_pending.append(2 * (NSQT - 1))
                emit_ln(ln_pending.pop(0))
                emit_ln(ln_pending.pop(0))
            elif c == 3:
                ln_pending.append(2 * (NSQT - 1) + 1)
        while ln_pending:
            emit_ln(ln_pending.pop(0))

    nc.compile()
    return nc


def _prep_inputs(x, Wq, bq, Wk, bk, Wv, bv, Wo, bo, gamma, beta):
    """Shard + lay out the full inputs for the 8 cores."""
    f32 = np.float32
    x = np.asarray(x, f32)
    Wq, bq = np.asarray(Wq, f32), np.asarray(bq, f32)
    Wk, bk = np.asarray(Wk, f32), np.asarray(bk, f32)
    Wv, bv = np.asarray(Wv, f32), np.asarray(bv, f32)
    Wo, bo = np.asarray(Wo, f32), np.asarray(bo, f32)
    gamma, beta = np.asarray(gamma, f32), np.asarray(beta, f32)

    mask = np.triu(np.ones((128, 128), f32)).astype(BFNP)
    vone = np.ones((128, 8), BFNP)
    one1 = np.ones((1, 128), BFNP)
    gmb = np.ascontiguousarray(np.broadcast_to(gamma, (128, HID)))
    btb = np.ascontiguousarray(np.broadcast_to(beta, (128, HID)))

    halves = []
    for h in range(2):
        sl = slice(HW * h, HW * (h + 1))
        halves.append(dict(
            wq16=np.ascontiguousarray(Wq.T[:, sl]).astype(BFNP),
            wk16=np.ascontiguousarray(Wk.T[:, sl]).astype(BFNP),
            wv16=np.ascontiguousarray(Wv.T[:, sl]).astype(BFNP),
            wo16=np.ascontiguousarray(Wo[:, sl].T).astype(BFNP),
            bq4=np.ascontiguousarray(bq[sl].reshape(4, 128).T),
            bk4=np.ascontiguousarray(bk[sl].reshape(4, 128).T),
            bv1=np.ascontiguousarray(bv[sl].reshape(1, HW)).astype(BFNP),
        ))

    in_maps = []
    for c in range(N_CORES):
        b, h = c // 2, c % 2
        m = dict(halves[h])
        m["xT16"] = np.ascontiguousarray(x[b].T).astype(BFNP)
        # rows this core receives from the chunked pairwise RS:
        # chunk k covers global rows [256k + 128h, 256k + 128h + 128)
        m["xh"] = np.ascontiguousarray(
            np.concatenate([x[b, 256 * k + 128 * h:256 * k + 128 * h + 128, :]
                            for k in range(8)], axis=0) + bo)
        m["gmb"] = gmb
        m["btb"] = btb
        m["m128"] = mask
        m["vone"] = vone
        m["one1"] = one1
        in_maps.append(m)
    return in_maps


def _run(inputs, trace=False):
    gamma = np.asarray(inputs["gamma"], np.float32)
    beta = np.asarray(inputs["beta"], np.float32)
    apply_gb = not (np.allclose(gamma, 1.0) and np.allclose(beta, 0.0))
    key = ("nc", apply_gb)
    if key not in _CACHE:
        _CACHE[key] = _build(apply_gb)
    nc = _CACHE[key]
    in_maps = _prep_inputs(**inputs)
    res = run_bass_kernel_spmd(nc, in_maps, list(range(N_CORES)),
                               trace=trace)
    out = np.empty((B, S, HID), np.float32)
    for c in range(N_CORES):
        b, h = c // 2, c % 2
        o = res.results[c]["out"]
        for k in range(8):
            out[b, 256 * k + 128 * h:256 * k + 128 * h + 128, :] = \
                o[128 * k:128 * (k + 1), :]
    return out, res


def kernel(**inputs):
    out, _ = _run(inputs, trace=False)
    return out


# revision 7
# speedup vs baseline: 1.3311x; 1.0045x over previous
"""Causal self-attention block (QKV proj + causal MHA + out proj + residual
+ LayerNorm) for B=4, S=2048, HID=1024, 16 heads, on 8 Trainium2 cores.

Sharding: core c handles batch b=c//2 and heads [8h, 8h+8) where h=c%2
(Megatron-style head split within a batch pair). Each core computes its 8
heads' attention and a partial output projection over the full 2048 rows;
the two cores of a batch pair combine partials with pairwise bf16
ReduceScatters (chunked, pipelined with compute; the final tile uses 4
finer chunks to drain the tail), then each core applies residual +
LayerNorm to its quarter-rows and returns [1024, 1024].

All matmuls run in bf16 (fp32 PSUM accumulation). Attention uses the
transposed-score layout (scoresT[sk, sq]): softmax sums fall out of the
PV matmul via an appended ones-row on V, causal structure shrinks
above-diagonal tiles, and each head pair shares fused two-bank PSUM
tiles so one ACT exp covers both heads; the score matmul for tile i+1 is
emitted ahead of PV(i) so the PE never waits on the exp. The Scalar
engine runs only Exp/Identity/Copy (single activation table, no
reloads); the LN rsqrt is computed on the Vector engine via
reciprocal seed + Newton iterations. Projection work for tile t+1 and
the out projection for tile j-1 are interleaved into attention tile j's
emission to keep the PE dense (p-state) and busy during softmax
normalization windows; LayerNorm chunks are deferred until well after
their ReduceScatter fires, use per-chunk scatter tensors (exact deps),
and all LN DMAs ride the sync queue so collective latency never blocks
the gpsimd queue feeding attention.
"""

import numpy as np
import ml_dtypes

import concourse.bacc as bacc
import concourse.mybir as mybir
import concourse.tile as tile
from concourse.bass_utils import run_bass_kernel_spmd

F32 = mybir.dt.float32
BF16 = mybir.dt.bfloat16
AF = mybir.ActivationFunctionType
OP = mybir.AluOpType
BFNP = ml_dtypes.bfloat16

N_CORES = 8
B, S, HID = 4, 2048, 1024
NHC = 8          # heads per core
DH = 64          # head dim
HW = 512         # per-core head width (NHC * DH)
SQT = 512        # sq tile width
NSQT = S // SQT  # 4
NHCH = HID // 128  # 8 hid chunks
SH = S // 2      # rows per core in the epilogue
EPS = 1e-5
GROUPS = [[0, 1], [2, 3], [4, 5], [6, 7]]

_CACHE = {}


def _build(apply_gb):
    nc = bacc.Bacc("TRN2", target_bir_lowering=False, debug=False,
                   num_devices=N_CORES)

    xst_d = [nc.dram_tensor(f"xst{t}", [128, 8 * SQT], BF16,
                            kind="ExternalInput").ap() for t in range(NSQT)]
    xh = nc.dram_tensor("xh", [SH, HID], F32, kind="ExternalInput").ap()
    wqs_d = nc.dram_tensor("wqs", [128, 8 * HW], BF16,
                           kind="ExternalInput").ap()
    wks_d = nc.dram_tensor("wks", [128, 8 * HW], BF16,
                           kind="ExternalInput").ap()
    wvs_d = nc.dram_tensor("wvs", [128, 8 * HW], BF16,
                           kind="ExternalInput").ap()
    wos_d = nc.dram_tensor("wos", [128, 4 * HID], BF16,
                           kind="ExternalInput").ap()
    bq4 = nc.dram_tensor("bq4", [128, 4], F32, kind="ExternalInput").ap()
    bk4 = nc.dram_tensor("bk4", [128, 4], F32, kind="ExternalInput").ap()
    bv1 = nc.dram_tensor("bv1", [1, HW], BF16, kind="ExternalInput").ap()
    one1 = nc.dram_tensor("one1", [1, 128], BF16, kind="ExternalInput").ap()
    vone = nc.dram_tensor("vone", [128, 8], BF16, kind="ExternalInput").ap()
    m128 = nc.dram_tensor("m128", [128, 128], BF16, kind="ExternalInput").ap()
    gmb = nc.dram_tensor("gmb", [128, HID], F32, kind="ExternalInput").ap()
    btb = nc.dram_tensor("btb", [128, HID], F32, kind="ExternalInput").ap()

    out = nc.dram_tensor("out", [SH, HID], F32, kind="ExternalOutput").ap()

    po_d = nc.dram_tensor("po_d", [S, HID], BF16).ap()
    # per-chunk scatter outputs so LayerNorm dma deps are exact
    rsd = [nc.dram_tensor(f"rs{k}", [128, HID], BF16).ap() for k in range(6)]
    rs3 = [nc.dram_tensor(f"rs3_{c}", [64, HID], BF16).ap() for c in range(4)]

    from contextlib import ExitStack
    with tile.TileContext(nc) as tc, ExitStack() as es:
        TP = tc.tile_pool
        cp = es.enter_context(TP(name="consts", bufs=1))
        xsp = es.enter_context(TP(name="xs", bufs=1))
        wp = es.enter_context(TP(name="w", bufs=1))
        ktp = es.enter_context(TP(name="kt", bufs=1))
        vtp = es.enter_context(TP(name="vt", bufs=1))
        qtp = es.enter_context(TP(name="qt", bufs=2))
        ep = es.enter_context(TP(name="exp", bufs=2))
        atp = es.enter_context(TP(name="att", bufs=2))
        pop = es.enter_context(TP(name="po", bufs=2))
        rp = es.enter_context(TP(name="rcp", bufs=2))
        rbp = es.enter_context(TP(name="rb", bufs=2))
        lp = es.enter_context(TP(name="ln", bufs=2))
        lsp = es.enter_context(TP(name="lns", bufs=2))
        pp = es.enter_context(TP(name="pp", bufs=2, space="PSUM"))
        sp = es.enter_context(TP(name="sp", bufs=2, space="PSUM"))
        app = es.enter_context(TP(name="ap", bufs=1, space="PSUM"))

        # ---- staged preload: one DMA per weight group / x tile, spread
        # across queues so issue cost doesn't serialize ----
        wqs = wp.tile([128, 8 * HW], BF16, name="wqs")
        nc.sync.dma_start(wqs[:], wqs_d[:])
        xst = [xsp.tile([128, 8 * SQT], BF16, name=f"xst{t}")
               for t in range(NSQT)]
        nc.gpsimd.dma_start(xst[0][:], xst_d[0][:])
        wks = wp.tile([128, 8 * HW], BF16, name="wks")
        nc.scalar.dma_start(wks[:], wks_d[:])
        nc.gpsimd.dma_start(xst[1][:], xst_d[1][:])
        wvs = wp.tile([128, 8 * HW], BF16, name="wvs")
        nc.sync.dma_start(wvs[:], wvs_d[:])
        nc.sync.dma_start(xst[2][:], xst_d[2][:])
        wos = wp.tile([128, 4 * HID], BF16, name="wos")
        nc.scalar.dma_start(wos[:], wos_d[:])
        nc.gpsimd.dma_start(xst[3][:], xst_d[3][:])

        # ---- constants ----
        mask = cp.tile([128, 128], BF16)
        nc.sync.dma_start(mask[:], m128[:])
        bqs = cp.tile([128, 4], F32)
        nc.sync.dma_start(bqs[:], bq4[:])
        bks = cp.tile([128, 4], F32)
        nc.sync.dma_start(bks[:], bk4[:])
        bvs = cp.tile([1, HW], BF16)
        nc.sync.dma_start(bvs[:], bv1[:])
        o1s = cp.tile([1, 128], BF16)
        nc.sync.dma_start(o1s[:], one1[:])
        vos = cp.tile([128, 8], BF16)
        nc.sync.dma_start(vos[:], vone[:])
        epsc = cp.tile([128, 1], F32)
        nc.vector.memset(epsc[:], EPS)
        if apply_gb:
            gms = cp.tile([128, HID], F32)
            nc.sync.dma_start(gms[:], gmb[:])
            bts = cp.tile([128, HID], F32)
            nc.sync.dma_start(bts[:], btb[:])

        kt = [ktp.tile([128, S], BF16, name=f"kt{p}") for p in range(4)]
        vt = [vtp.tile([128, 8, 65], BF16, name=f"vt{i}") for i in range(16)]
        for i in range(16):
            nc.vector.tensor_copy(
                vt[i][:, :, 64:65],
                vos[:].rearrange("p (a b) -> p a b", a=8))

        qts_map = {}
        at_map = {}

        def wsl(ws, hh):
            return ws[:, HW * hh:HW * (hh + 1)]

        def xsl(t, hh, c0, w):
            return xst[t][:, SQT * hh + c0:SQT * hh + c0 + w]

        # ---- phase-A units: projections for sq tile t ----
        def unit_q(t, m):
            ps = pp.tile([128, SQT], F32, tag="pq")
            for hh in range(NHCH):
                nc.tensor.matmul(
                    ps[:], wsl(wqs, hh)[:, 128 * m:128 * (m + 1)],
                    xsl(t, hh, 0, SQT),
                    start=(hh == 0), stop=(hh == NHCH - 1))
            qt_ = qtp.tile([128, SQT], BF16, tag=f"q{m}")
            nc.scalar.activation(qt_[:], ps[:], AF.Identity,
                                 bias=bqs[:, m:m + 1])
            qts_map[(t, m)] = qt_

        def unit_k(t, m):
            ps = pp.tile([128, SQT], F32, tag="pq")
            for hh in range(NHCH):
                nc.tensor.matmul(
                    ps[:], wsl(wks, hh)[:, 128 * m:128 * (m + 1)],
                    xsl(t, hh, 0, SQT),
                    start=(hh == 0), stop=(hh == NHCH - 1))
            nc.scalar.activation(kt[m][:, SQT * t:SQT * (t + 1)], ps[:],
                                 AF.Identity, bias=bks[:, m:m + 1])

        def unit_v(t, s_):
            i = 4 * t + s_
            ps = pp.tile([128, HW], F32, tag="pq")
            for hh in range(NHCH):
                nc.tensor.matmul(
                    ps[:], xsl(t, hh, 128 * s_, 128), wsl(wvs, hh),
                    start=(hh == 0), stop=False)
            nc.tensor.matmul(ps[:], o1s[:], bvs[:], start=False, stop=True)
            nc.scalar.activation(
                vt[i][:, :, 0:64],
                ps[:].rearrange("p (a b) -> p a b", a=8), AF.Copy)

        def a_units(t):
            us = []
            for m in range(4):
                us.append(lambda m=m: unit_k(t, m))
            for m in range(4):
                us.append(lambda m=m: unit_q(t, m))
            for s_ in range(4):
                us.append(lambda s_=s_: unit_v(t, s_))
            return us

        # ---- partial out projection for row chunk c of sq tile j ----
        def emit_outproj(j, c):
            at_tiles = [at_map[(j, p)] for p in range(4)]
            po = pop.tile([128, HID], BF16, tag="po")
            for o in range(2):
                ps = pp.tile([128, SQT], F32, tag="pq")
                for dch in range(4):
                    nc.tensor.matmul(
                        ps[:], at_tiles[dch][:, 128 * c:128 * (c + 1)],
                        wos[:, HID * dch + SQT * o:
                            HID * dch + SQT * (o + 1)],
                        start=(dch == 0), stop=(dch == 3))
                nc.vector.tensor_copy(po[:, SQT * o:SQT * (o + 1)], ps[:])
            r0 = SQT * j + 128 * c
            nc.sync.dma_start(po_d[r0:r0 + 128, :], po[:])
            if j < NSQT - 1:
                if c in (1, 3):
                    h0 = SQT * j + 256 * (c // 2)
                    k = 2 * j + c // 2
                    nc.gpsimd.collective_compute(
                        "ReduceScatter", OP.add, replica_groups=GROUPS,
                        ins=[po_d[h0:h0 + 256, :]],
                        outs=[rsd[k][:]])
            else:
                nc.gpsimd.collective_compute(
                    "ReduceScatter", OP.add, replica_groups=GROUPS,
                    ins=[po_d[r0:r0 + 128, :]],
                    outs=[rs3[c][:]])

        # ---- residual + LayerNorm for a pair of output chunks ----
        def ln_load(k):
            rs = lp.tile([128, HID], BF16, tag="rs")
            if k < 6:
                nc.sync.dma_start(rs[:], rsd[k][:])
            else:
                nc.sync.dma_start(rs[0:64, :], rs3[2 * (k - 6)][:])
                nc.sync.dma_start(rs[64:128, :], rs3[2 * (k - 6) + 1][:])
            xc = lp.tile([128, HID], F32, tag="xc")
            nc.sync.dma_start(xc[:], xh[128 * k:128 * (k + 1), :])
            y = lp.tile([128, HID], F32, tag="y")
            nc.vector.tensor_tensor(y[:], rs[:], xc[:], op=OP.add)
            st6 = lsp.tile([128, 12], F32, tag="st6")
            nc.vector.bn_stats(st6[:, 0:6], y[:, 0:512])
            nc.vector.bn_stats(st6[:, 6:12], y[:, 512:1024])
            mv = lsp.tile([128, 2], F32, tag="mv")
            nc.vector.bn_aggr(mv[:], st6[:])
            return rs, xc, y, mv

        def emit_ln_pair(k0):
            a = ln_load(k0)
            b = ln_load(k0 + 1)
            ve = lsp.tile([128, 2], F32, tag="ve")
            nc.vector.tensor_scalar_add(ve[:, 0:1], a[3][:, 1:2], epsc[:])
            nc.vector.tensor_scalar_add(ve[:, 1:2], b[3][:, 1:2], epsc[:])
            # 1/sqrt(ve) on DVE: 1/ve seed + 4 Newton iterations
            ry = lsp.tile([128, 2], F32, tag="ry")
            nc.vector.reciprocal_approx_fast(ry[:], ve[:])
            tmp = lsp.tile([128, 2], F32, tag="tmp")
            for _ in range(4):
                nc.vector.tensor_mul(tmp[:], ry[:], ry[:])
                nc.vector.tensor_mul(tmp[:], tmp[:], ve[:])
                nc.vector.tensor_scalar(tmp[:], tmp[:], -0.5, 1.5,
                                        op0=OP.mult, op1=OP.add)
                nc.vector.tensor_mul(ry[:], ry[:], tmp[:])
            for idx, (rs, xc, y, mv) in enumerate((a, b)):
                nc.vector.tensor_scalar(xc[:], y[:], mv[:, 0:1],
                                        ry[:, idx:idx + 1],
                                        op0=OP.subtract, op1=OP.mult)
                if apply_gb:
                    nc.vector.tensor_mul(xc[:], xc[:], gms[:])
                    nc.vector.tensor_add(xc[:], xc[:], bts[:])
                k = k0 + idx
                nc.sync.dma_start(out[128 * k:128 * (k + 1), :], xc[:])

        # ---- attention p-group for sq tile j ----
        def emit_attn_p(j, p):
            qt_ = qts_map[(j, p)]
            pv2 = app.tile([65, 2 * SQT], F32, tag="pv2")
            last = 4 * j + 3
            pend = None
            for i in range(4 * j + 4):
                d = i - 4 * j
                lo = 128 * d if d >= 0 else 0
                s2 = sp.tile([128, 2 * SQT], F32, tag="s2")
                nc.tensor.matmul(
                    s2[:, lo:SQT],
                    kt[p][0:64, 128 * i:128 * (i + 1)],
                    qt_[0:64, lo:SQT],
                    start=True, stop=True, tile_position=(0, 0))
                nc.tensor.matmul(
                    s2[:, SQT + lo:2 * SQT],
                    kt[p][64:128, 128 * i:128 * (i + 1)],
                    qt_[64:128, lo:SQT],
                    start=True, stop=True, tile_position=(64, 0))
                e2 = ep.tile([128, 2 * SQT], BF16, tag="e2")
                s2v = s2[:].rearrange("p (a b) -> p a b", a=2)
                e2v = e2[:].rearrange("p (a b) -> p a b", a=2)
                nc.scalar.activation(e2v[:, :, lo:SQT], s2v[:, :, lo:SQT],
                                     AF.Exp, scale=0.125)
                if d >= 0:
                    nc.vector.tensor_mul(
                        e2[:, lo:lo + 128], e2[:, lo:lo + 128], mask[:])
                    nc.vector.tensor_mul(
                        e2[:, SQT + lo:SQT + lo + 128],
                        e2[:, SQT + lo:SQT + lo + 128], mask[:])
                if pend is not None:
                    pl, pe2 = pend
                    nc.tensor.matmul(
                        pv2[:, pl:SQT], vt[i - 1][:, 2 * p, :],
                        pe2[:, pl:SQT], start=(i - 1 == 0), stop=False)
                    nc.tensor.matmul(
                        pv2[:, SQT + pl:2 * SQT], vt[i - 1][:, 2 * p + 1, :],
                        pe2[:, SQT + pl:2 * SQT],
                        start=(i - 1 == 0), stop=False)
                pend = (lo, e2)
            pl, pe2 = pend
            nc.tensor.matmul(
                pv2[:, pl:SQT], vt[last][:, 2 * p, :],
                pe2[:, pl:SQT], start=(last == 0), stop=True)
            nc.tensor.matmul(
                pv2[:, SQT + pl:2 * SQT], vt[last][:, 2 * p + 1, :],
                pe2[:, SQT + pl:2 * SQT],
                start=(last == 0), stop=True)
            sm = rp.tile([1, 2 * SQT], F32, tag="sm")
            nc.vector.tensor_copy(sm[:], pv2[64:65, :])
            rc = rp.tile([1, 2 * SQT], F32, tag="rc")
            nc.vector.reciprocal_approx_fast(rc[:], sm[:])
            rb = rbp.tile([64, 2 * SQT], F32, tag="rb")
            nc.gpsimd.partition_broadcast(rb[:], rc[:])
            at_ = atp.tile([128, SQT], BF16, tag=f"at{p}")
            nc.vector.tensor_tensor(at_[0:64, :], pv2[0:64, 0:SQT],
                                    rb[:, 0:SQT], op=OP.mult)
            nc.vector.tensor_tensor(at_[64:128, :], pv2[0:64, SQT:2 * SQT],
                                    rb[:, SQT:2 * SQT], op=OP.mult)
            at_map[(j, p)] = at_

        # ---- emission schedule ----
        for u in a_units(0):
            u()
        for j in range(NSQT):
            nxt = a_units(j + 1) if j + 1 < NSQT else []
            for p in range(4):
                emit_attn_p(j, p)
                for u in nxt[3 * p:3 * p + 3]:
                    u()
                if p == 1 and j >= 2:
                    emit_ln_pair(2 * (j - 2))
                if j >= 1:
                    if p == 2:
                        emit_outproj(j - 1, 0)
                        emit_outproj(j - 1, 1)
                    elif p == 3:
                        emit_outproj(j - 1, 2)
                        emit_outproj(j - 1, 3)
        for c in range(4):
            emit_outproj(NSQT - 1, c)
            if c == 1:
                emit_ln_pair(2 * (NSQT - 2))
        emit_ln_pair(2 * (NSQT - 1))

    nc.compile()
    return nc


def _prep_inputs(x, Wq, bq, Wk, bk, Wv, bv, Wo, bo, gamma, beta):
    """Shard + lay out the full inputs for the 8 cores."""
    f32 = np.float32
    x = np.asarray(x, f32)
    Wq, bq = np.asarray(Wq, f32), np.asarray(bq, f32)
    Wk, bk = np.asarray(Wk, f32), np.asarray(bk, f32)
    Wv, bv = np.asarray(Wv, f32), np.asarray(bv, f32)
    Wo, bo = np.asarray(Wo, f32), np.asarray(bo, f32)
    gamma, beta = np.asarray(gamma, f32), np.asarray(beta, f32)

    mask = np.triu(np.ones((128, 128), f32)).astype(BFNP)
    vone = np.ones((128, 8), BFNP)
    one1 = np.ones((1, 128), BFNP)
    gmb = np.ascontiguousarray(np.broadcast_to(gamma, (128, HID)))
    btb = np.ascontiguousarray(np.broadcast_to(beta, (128, HID)))

    def stage_w(WT):
        # [1024, 512] -> [128, 8*512] with col block hh = rows 128hh
        return np.ascontiguousarray(
            WT.reshape(8, 128, HW).transpose(1, 0, 2).reshape(128, 8 * HW)
        ).astype(BFNP)

    halves = []
    for h in range(2):
        sl = slice(HW * h, HW * (h + 1))
        woT = Wo[:, sl].T  # [512, 1024]
        halves.append(dict(
            wqs=stage_w(np.ascontiguousarray(Wq.T[:, sl])),
            wks=stage_w(np.ascontiguousarray(Wk.T[:, sl])),
            wvs=stage_w(np.ascontiguousarray(Wv.T[:, sl])),
            wos=np.ascontiguousarray(
                woT.reshape(4, 128, HID).transpose(1, 0, 2)
                .reshape(128, 4 * HID)).astype(BFNP),
            bq4=np.ascontiguousarray(bq[sl].reshape(4, 128).T),
            bk4=np.ascontiguousarray(bk[sl].reshape(4, 128).T),
            bv1=np.ascontiguousarray(bv[sl].reshape(1, HW)).astype(BFNP),
        ))

    def row_blocks(h):
        # output chunk k -> list of (global row start, nrows)
        blocks = []
        for k in range(6):
            blocks.append([(256 * k + 128 * h, 128)])
        blocks.append([(1536 + 64 * h, 64), (1664 + 64 * h, 64)])
        blocks.append([(1792 + 64 * h, 64), (1920 + 64 * h, 64)])
        return blocks

    in_maps = []
    for c in range(N_CORES):
        b, h = c // 2, c % 2
        m = dict(halves[h])
        xT = np.ascontiguousarray(x[b].T).astype(BFNP)  # [1024, 2048]
        # [1024, 2048] -> per tile t: [128, 8*512], col block hh = rows 128hh
        xr = xT.reshape(8, 128, NSQT, SQT)
        for t in range(NSQT):
            m[f"xst{t}"] = np.ascontiguousarray(
                xr[:, :, t, :].transpose(1, 0, 2).reshape(128, 8 * SQT))
        m["xh"] = np.ascontiguousarray(np.concatenate(
            [x[b, r0:r0 + n, :] for blk in row_blocks(h)
             for (r0, n) in blk], axis=0) + bo)
        m["gmb"] = gmb
        m["btb"] = btb
        m["m128"] = mask
        m["vone"] = vone
        m["one1"] = one1
        in_maps.append(m)
    return in_maps


def _run(inputs, trace=False):
    gamma = np.asarray(inputs["gamma"], np.float32)
    beta = np.asarray(inputs["beta"], np.float32)
    apply_gb = not (np.allclose(gamma, 1.0) and np.allclose(beta, 0.0))
    key = ("nc", apply_gb)
    if key not in _CACHE:
        _CACHE[key] = _build(apply_gb)
    nc = _CACHE[key]
    in_maps = _prep_inputs(**inputs)
    res = run_bass_kernel_spmd(nc, in_maps, list(range(N_CORES)),
                               trace=trace)
    out = np.empty((B, S, HID), np.float32)
    for c in range(N_CORES):
        b, h = c // 2, c % 2
        o = res.results[c]["out"]
        row = 0
        for k in range(6):
            out[b, 256 * k + 128 * h:256 * k + 128 * h + 128, :] = \
                o[row:row + 128, :]
            row += 128
        for r0 in (1536 + 64 * h, 1664 + 64 * h, 1792 + 64 * h,
                   1920 + 64 * h):
            out[b, r0:r0 + 64, :] = o[row:row + 64, :]
            row += 64
    return out, res


def kernel(**inputs):
    out, _ = _run(inputs, trace=False)
    return out
